# revision 39
# baseline (speedup 1.0000x reference)
"""HDTimeCrystalBlock kernel for 8 Trainium2 NeuronCores.

Math: out = ((x @ W_in) * mod[None]) @ W_out, where
  mod[l,h] = sum_m coupled[m] * cos(omega*(m+1)*t[l] + E[m,h])
With cos(a+b) = cos(a)cos(b) - sin(a)sin(b), mod is a K=32 matmul:
  mod[h,l] = sum_r ab[r,h] * cs[r,l]
  ab rows 0:16  = coupled[m]*cos(E[m,h]),  rows 16:32 = -coupled[m]*sin(E[m,h])
  cs rows 0:16  = cos(omega*(m+1)*t[l]),   rows 16:32 = sin(...)
ab/cs are tiny and precomputed on the HOST.  Both are shipped with the 32
rows duplicated at partitions 32:64 so consecutive j-tiles' mod matmuls
run CONCURRENTLY in different 32-row strips of the PE array (row tiling).

Sharding: data-parallel over the 8192 tokens (B*L), 1024 per core; weights
replicated.  All matmul operands bf16 (1 col/cycle @ 2.4 GHz), bf16 output.

Data movement (the hard-won part):
  - The 16 DMA engines serve ALL outstanding transfers round-robin and are
    packet-rate bound (~125 pkts/us early), so (a) every tensor is
    pre-permuted ON THE HOST into partition-major layout so each DMA moves
    2-8KB contiguous runs per partition, and (b) later chunks must not be
    in flight while the critical set loads.  Priority is enforced with
    dependency CHAINS: each later chunk's destination tile is pre-written
    with a 1-element copy reading an earlier tile, so its DMA (WAW) cannot
    start before the earlier chunk (RAW) has landed.  Two chained streams
    (W_in on sync, W_out+xts1 on gpsimd) follow consumption order.
  - PE warm-up matmuls on a memset scratch tile keep the PE busy from
    ~7.5us so HAM un-throttles to 2.4 GHz right as the critical set lands.
Steady loop: j-pairs, mm2 software-pipelined one pair behind; mod copy on
Act, hd*mod multiply on DVE; the previous q's outputs drain at pair 1 of
the next q; the last q's drain chases the final flush matmuls.
"""
import math

import numpy as np

B, L, D, HD, M = 4, 2048, 512, 4096, 16
NCORES = 8
T = (B * L) // NCORES          # tokens per core
QCH = 512                      # l-chunk (PSUM bank width in fp32)
HCH = QCH // 2
NQ = T // QCH
NJ = HD // 128                 # h-tiles
NK = D // 128                  # d-tiles
K2 = 2 * M                     # mod-matmul contraction (32)
NWARM = 10                     # scratch warm-up matmuls (N=512)

# j-tile ranges per DMA chunk for w_in / w_out ([lo, hi) in j-tiles)
WIN_PARTS = [(0, 1), (1, 2), (2, 4), (4, 8), (8, 12), (12, 16), (16, 20),
             (20, 24), (24, 28), (28, 32)]
WOUT_PARTS = [(0, 4), (4, 8), (8, 12), (12, 16), (16, 20),
              (20, 24), (24, 28), (28, 32)]

_cache = {}


def _build():
    from concourse import bacc, bass, mybir, tile

    F32 = mybir.dt.float32
    BF16 = mybir.dt.bfloat16
    PSUM = bass.MemorySpace.PSUM

    nc = bacc.Bacc("TRN2", target_bir_lowering=False, debug=False)

    # host-permuted, partition-major layouts (2-8KB contiguous per row):
    #   xT2[p, q*NK*512 + k*512 + t'] = x[q*512+t', k*128+p]
    #   wi2[p, (j*NK+k)*128 + c]      = w_in[k*128+p, j*128+c]
    #   wo2[p, j*512 + i]             = w_out[j*128+p, i]
    xT_d = nc.dram_tensor("xT", [128, NQ * NK * QCH], BF16, kind="ExternalInput")
    wi_d = nc.dram_tensor("wi", [128, NJ * NK * 128], BF16, kind="ExternalInput")
    wo_d = nc.dram_tensor("wo", [128, NJ * D], BF16, kind="ExternalInput")
    cs_d = nc.dram_tensor("cs", [K2, T], BF16, kind="ExternalInput")
    ab_d = nc.dram_tensor("ab", [K2, HD], BF16, kind="ExternalInput")
    yT_d = nc.dram_tensor("yT", [D, T], BF16, kind="ExternalOutput")

    with tile.TileContext(nc) as tc:
        with (
            tc.tile_pool(name="win", bufs=1) as winp,
            tc.tile_pool(name="wout", bufs=1) as woutp,
            tc.tile_pool(name="xts", bufs=1) as xtp,
            tc.tile_pool(name="small", bufs=1) as smallp,
            tc.tile_pool(name="hm", bufs=4) as hmp,
            tc.tile_pool(name="mods", bufs=4) as modsp,
            tc.tile_pool(name="yo", bufs=4) as yop,
            tc.tile_pool(name="pa", bufs=2, space=PSUM) as pap,
            tc.tile_pool(name="pb", bufs=2, space=PSUM) as pbp,
            tc.tile_pool(name="py", bufs=4, space=PSUM) as pyp,
        ):
            # ---- scratch warm-up tile, memset on the otherwise-idle DVE ----
            wmt = smallp.tile([128, QCH], BF16, tag="wmt")
            nc.vector.memset(wmt[:], 0.5)

            ab = smallp.tile([2 * K2, HD], BF16, tag="ab")
            cs = smallp.tile([2 * K2, T], BF16, tag="cs")

            win_c = [None] * len(WIN_PARTS)
            wout_g = [None] * len(WOUT_PARTS)
            xts_q = [None] * NQ

            def win_tile(i):
                a, b = WIN_PARTS[i]
                t_ = winp.tile([128, (b - a) * NK * 128], BF16,
                               name=f"win{i}", tag=f"win{i}")
                win_c[i] = t_
                return t_

            def wout_tile(i):
                a, b = WOUT_PARTS[i]
                tw = woutp.tile([128, (b - a) * D], BF16,
                                name=f"wout{i}", tag=f"wout{i}")
                wout_g[i] = tw
                return tw

            def win_dma(i):
                a, b = WIN_PARTS[i]
                nc.sync.dma_start(win_c[i][:],
                                  wi_d[:, a * NK * 128 : b * NK * 128])

            def wout_dma(i):
                a, b = WOUT_PARTS[i]
                nc.gpsimd.dma_start(wout_g[i][:], wo_d[:, a * D : b * D])

            def chain(dst_tile, src_tile):
                # 1-elem pre-write of dst reading src: dst's DMA (WAW) must
                # follow src's landing (RAW); the real DMA then overwrites
                nc.gpsimd.tensor_copy(dst_tile[0:1, 0:1], src_tile[0:1, 0:1])

            # ---- critical set: win j0-1, xts0, ab, cs -- nothing else is
            # allowed in flight until these land ----
            tx0 = xtp.tile([128, NK * QCH], BF16, name="xts0", tag="xts0")
            xts_q[0] = tx0
            win_tile(0)
            win_dma(0)
            nc.scalar.dma_start(tx0[:], xT_d[:, 0 : NK * QCH])
            nc.gpsimd.dma_start(ab[0:K2, :], ab_d[:])
            nc.scalar.dma_start(cs[0:K2, :], cs_d[:])
            # duplicate the 32 ab/cs rows at partitions 32:64 on-device
            # (fast SBUF->SBUF DMA) so they aren't critical-path HBM bytes
            nc.gpsimd.dma_start(ab[K2 : 2 * K2, :], ab[0:K2, :])
            nc.scalar.dma_start(cs[K2 : 2 * K2, :], cs[0:K2, :])

            # ---- chained streams in consumption order (gated on the whole
            # critical set: xts0 + ab + cs) ----
            t_ = win_tile(1)
            chain(t_, tx0)
            # read from the dup rows so streams also wait for the on-device
            # ab/cs duplication, which must not be starved
            nc.gpsimd.tensor_copy(t_[0:1, 0:1], ab[K2 : K2 + 1, 0:1])
            nc.gpsimd.tensor_copy(t_[0:1, 0:1], cs[K2 : K2 + 1, 0:1])
            win_dma(1)
            tw = wout_tile(0)
            chain(tw, tx0)
            nc.gpsimd.tensor_copy(tw[0:1, 0:1], ab[K2 : K2 + 1, 0:1])
            wout_dma(0)
            t_ = win_tile(2)
            chain(t_, win_c[1])
            win_dma(2)
            tw = wout_tile(1)
            chain(tw, wout_g[0])
            wout_dma(1)
            tx1 = xtp.tile([128, NK * QCH], BF16, name="xts1", tag="xts1")
            xts_q[1] = tx1
            chain(tx1, wout_g[0])
            nc.gpsimd.dma_start(tx1[:], xT_d[:, NK * QCH : 2 * NK * QCH])
            for i in range(3, len(WIN_PARTS)):
                t_ = win_tile(i)
                chain(t_, win_c[i - 1])
                win_dma(i)
                if i - 1 < len(WOUT_PARTS):
                    tw = wout_tile(i - 1)
                    chain(tw, wout_g[i - 2])
                    wout_dma(i - 1)
            for i in range(len(WIN_PARTS) - 1, len(WOUT_PARTS)):
                tw = wout_tile(i)
                chain(tw, wout_g[i - 1])
                wout_dma(i)

            def win_slice(j, k):
                for i, (a, b) in enumerate(WIN_PARTS):
                    if a <= j < b:
                        off = ((j - a) * NK + k) * 128
                        return win_c[i][:, off : off + 128]
                raise AssertionError

            def wout_slice(j, j2):
                for i, (a, b) in enumerate(WOUT_PARTS):
                    if a <= j < b:
                        off = (j - a) * D + 128 * j2
                        return wout_g[i][:, off : off + 128]
                raise AssertionError

            def xts_slice(q, k):
                return xts_q[q][:, k * QCH : (k + 1) * QCH]

            # ---- PE warm-up on scratch (HAM to K=8/8 by ~11.5us) ----
            for w in range(NWARM):
                pw = pap.tile([128, QCH], F32, name=f"warm{w}", tag="pa")
                nc.tensor.matmul(pw[:], wmt[:, 0:128], wmt[:], start=True, stop=True)

            def emit_yo_batch(pq, ppys):
                yos = []
                for j2 in range(NK):
                    yo = yop.tile([128, QCH], BF16, name=f"yo{pq}_{j2}", tag="yo")
                    if j2 % 2 == 0:
                        nc.scalar.copy(yo[:], ppys[j2][:])
                    else:
                        nc.vector.tensor_copy(yo[:], ppys[j2][:])
                    yos.append(yo)
                for j2 in range(NK):
                    nc.sync.dma_start(
                        yT_d[128 * j2 : 128 * (j2 + 1),
                             pq * QCH : (pq + 1) * QCH],
                        yos[j2][:],
                    )

            # ---- fused main loop: j-pairs, mm2 pipelined one pair behind ----
            prev_q = None  # (q, pys) drained at pair 1 of the next q
            for q in range(NQ):
                lo, hi = q * QCH, (q + 1) * QCH
                pys = [pyp.tile([128, QCH], F32, name=f"py{q}_{j2}", tag="py")
                       for j2 in range(NK)]
                pend = None
                for p in range(NJ // 2):
                    j0, j1 = 2 * p, 2 * p + 1
                    pa0 = pap.tile([128, QCH], F32, tag="pa")
                    for k in range(NK):
                        nc.tensor.matmul(pa0[:], win_slice(j0, k),
                                         xts_slice(q, k),
                                         start=(k == 0), stop=(k == NK - 1))
                    pb0 = pbp.tile([128, QCH], F32, tag="pb")
                    nc.tensor.matmul(pb0[:], ab[0:K2, 128 * j0 : 128 * (j0 + 1)],
                                     cs[0:K2, lo:hi], start=True, stop=True)
                    pb1 = pbp.tile([128, QCH], F32, tag="pb")
                    nc.tensor.matmul(pb1[:],
                                     ab[K2 : 2 * K2, 128 * j1 : 128 * (j1 + 1)],
                                     cs[K2 : 2 * K2, lo:hi],
                                     start=True, stop=True)
                    msb0 = modsp.tile([128, QCH], F32, tag="mods")
                    nc.scalar.copy(msb0[:], pb0[:])
                    hm0 = hmp.tile([128, QCH], BF16, tag="hm")
                    nc.vector.tensor_mul(hm0[:], pa0[:], msb0[:])
                    pa1 = pap.tile([128, QCH], F32, tag="pa")
                    for k in range(NK):
                        nc.tensor.matmul(pa1[:], win_slice(j1, k),
                                         xts_slice(q, k),
                                         start=(k == 0), stop=(k == NK - 1))
                    msb1 = modsp.tile([128, QCH], F32, tag="mods")
                    nc.scalar.copy(msb1[:], pb1[:])
                    hm1 = hmp.tile([128, QCH], BF16, tag="hm")
                    nc.vector.tensor_mul(hm1[:], pa1[:], msb1[:])
                    # previous q's outputs drain here (after pair 0's msb/mul
                    # so the pa/pb recycle chain is never behind the copies)
                    if p == 1 and prev_q is not None:
                        pq, ppys = prev_q
                        emit_yo_batch(pq, ppys)
                        prev_q = None
                    if pend is not None:
                        for (pj, phm) in pend:
                            for j2 in range(NK):
                                nc.tensor.matmul(
                                    pys[j2][:],
                                    wout_slice(pj, j2),
                                    phm[:],
                                    start=(pj == 0),
                                    stop=(pj == NJ - 1),
                                )
                    pend = [(j0, hm0), (j1, hm1)]
                # flush last pair's mm2; for the final q the yo copies chase
                # the per-j2 stop matmuls so the tail is one copy deep
                last_q = q == NQ - 1
                yos = []
                for (pj, phm) in pend:
                    for j2 in range(NK):
                        nc.tensor.matmul(
                            pys[j2][:],
                            wout_slice(pj, j2),
                            phm[:],
                            start=(pj == 0),
                            stop=(pj == NJ - 1),
                        )
                        if last_q and pj == NJ - 1:
                            yo = yop.tile([128, QCH], BF16,
                                          name=f"yo{q}_{j2}", tag="yo")
                            if j2 % 2 == 0:
                                nc.scalar.copy(yo[:], pys[j2][:])
                            else:
                                nc.vector.tensor_copy(yo[:], pys[j2][:])
                            yos.append((j2, yo))
                engs = (nc.sync, nc.scalar, nc.gpsimd, nc.sync)
                for n_, (j2, yo) in enumerate(yos):
                    engs[n_].dma_start(
                        yT_d[128 * j2 : 128 * (j2 + 1), lo:hi], yo[:]
                    )
                prev_q = (q, pys)

    nc.finalize()
    return nc


def _get_nc():
    if "nc" not in _cache:
        _cache["nc"] = _build()
    return _cache["nc"]


def _bf16(a):
    import ml_dtypes
    return np.ascontiguousarray(np.asarray(a, dtype=np.float32).astype(ml_dtypes.bfloat16))


def _in_maps(x, input_proj, output_proj, floquet_energies, drive_weights,
             coupling_matrix):
    coupled = coupling_matrix.astype(np.float64) @ drive_weights.astype(np.float64)
    E = floquet_energies.astype(np.float64)
    ab = _bf16(np.concatenate(
        [coupled[:, None] * np.cos(E), -coupled[:, None] * np.sin(E)], axis=0
    ))
    # wi2[p, (j*NK+k)*128+c] = w_in[k*128+p, j*128+c]
    wi = _bf16(
        np.asarray(input_proj, dtype=np.float32)
        .reshape(NK, 128, NJ, 128).transpose(1, 2, 0, 3).reshape(128, -1)
    )
    # wo2[p, j*512+i] = w_out[j*128+p, i]
    wo = _bf16(
        np.asarray(output_proj, dtype=np.float32)
        .reshape(NJ, 128, D).transpose(1, 0, 2).reshape(128, -1)
    )

    harm = np.arange(1, M + 1, dtype=np.float64)
    maps = []
    for c in range(NCORES):
        b, half = c // 2, c % 2
        t = (half * T + np.arange(T, dtype=np.float64)) / L
        ang = 2.0 * np.pi * harm[:, None] * t[None, :]
        cs = _bf16(np.concatenate([np.cos(ang), np.sin(ang)], axis=0))
        # xT2[p, q*NK*512 + k*512 + t'] = x[q*512+t', k*128+p]
        xc = np.asarray(x[b, half * T : (half + 1) * T, :], dtype=np.float32)
        xT = _bf16(
            xc.reshape(NQ, QCH, NK, 128).transpose(3, 0, 2, 1).reshape(128, -1)
        )
        maps.append({"xT": xT, "wi": wi, "wo": wo, "cs": cs, "ab": ab})
    return maps


def kernel(x, input_proj, output_proj, floquet_energies, drive_weights,
           coupling_matrix, _trace=False, _trace_kwargs=None):
    from concourse.bass_utils import run_bass_kernel_spmd

    nc = _get_nc()
    maps = _in_maps(x, input_proj, output_proj, floquet_energies,
                    drive_weights, coupling_matrix)
    kw = dict(_trace_kwargs or {})
    res = run_bass_kernel_spmd(nc, maps, list(range(NCORES)), trace=_trace, **kw)
    out = np.empty((B, L, D), dtype=np.float32)
    for c in range(NCORES):
        b, half = c // 2, c % 2
        out[b, half * T : (half + 1) * T, :] = (
            res.results[c]["yT"].astype(np.float32).T
        )
    if _trace:
        return out, res
    return out
